# revision 4
# baseline (speedup 1.0000x reference)
"""DiagPooling (segment-reduce over square-image diagonals) on 8 NeuronCores.

Input  x: [8, 128, 512, 512] f32. Output: [8, 1, 513] f32 — per batch, the
mean over (channels, diagonal) of each diagonal offset in [-256, 256].

Sharding: batch b -> core b (data parallel, no communication).

Per-core algorithm (stride-513 trick): for a flat 512x512 image, element
(i, j) sits at a = 512*i + j = 513*i + (j - i). Viewing the flat image as
rows of 513 (an overlapping strided view), every diagonal becomes a COLUMN:
P[q, r] = flat[513*q + r] holds diagonal o = r (when q + r <= 511) or
o = r - 513 (when q + r >= 512). The wanted diagonals o in [-256, 256] are
exactly {r <= 256, q <= 511 - r} and {r >= 257, q >= 512 - r}; a fixed mask
folded with 1/(C*diag_len) turns the segment reduce into masked column sums.
Channels are summed first (the mask is channel-invariant), so the mask is
applied once to ~1 MiB instead of 128 MiB.

Banding: each 64-row band of P only needs a prefix [0, Plen) and a suffix
[SS, 513) of columns (the rest of the row lies on diagonals outside
[-256, 256]), so the DMA skips ~19% of HBM traffic. Unwanted elements that
do get loaded (band unions, merged holes) are zeroed by the mask.

SBUF layout: P row q = g*128 + p -> partition p, group g. Each group's live
columns are packed side by side (width 385/513/513/384, total 1795), which
also shortens the per-channel DVE accumulate.
"""

import numpy as np

import concourse.bass as bass
import concourse.bacc as bacc
import concourse.mybir as mybir
from concourse import tile
from concourse.bass_utils import run_bass_kernel_spmd

B, C, H = 8, 128, 512
R = H + 1               # 513: columns of the strided view
NG = 4                  # 512 q-rows -> 4 groups of 128 partitions
CH_ELEMS = H * H        # elements per (b, c) image
PAD = H                 # tail pad so the last P-row read stays in bounds
N_IN = C * CH_ELEMS + PAD
F32 = mybir.dt.float32

# Per 64-row subgroup s (q in [64s, 64s+64)): wanted columns are
# [0, SUB_P[s]) and [SUB_SS[s], 513).
SUB_P = [257, 257, 257, 257, 256, 192, 128, 64]
SUB_SS = [449, 385, 321, 257, 257, 257, 257, 257]
# Group unions (128 rows): prefix len / suffix start.
GRP_P = [max(SUB_P[2 * g], SUB_P[2 * g + 1]) for g in range(NG)]
GRP_SS = [min(SUB_SS[2 * g], SUB_SS[2 * g + 1]) for g in range(NG)]


def _group_segs():
    """Per group: list of (r0, length) packed consecutively. A group whose
    prefix and suffix (almost) touch is merged into one full segment."""
    segs = []
    for g in range(NG):
        if GRP_SS[g] <= GRP_P[g] + 1:
            segs.append([(0, R)])
        else:
            segs.append([(0, GRP_P[g]), (GRP_SS[g], R - GRP_SS[g])])
    return segs


GSEGS = _group_segs()
GRP_W = [sum(L for _, L in s) for s in GSEGS]       # [385, 513, 513, 384]
GRP_BASE = [sum(GRP_W[:g]) for g in range(NG)]
FP = sum(GRP_W)                                     # 1795 packed columns


def _col_of(g, r):
    """Packed column of (group g, column r). r must lie in a group segment."""
    off = GRP_BASE[g]
    for r0, L in GSEGS[g]:
        if r0 <= r < r0 + L:
            return off + (r - r0)
        off += L
    raise ValueError((g, r))


def _pieces(coarse: bool):
    """DMA piece list [(p0, np, r0, L, col0)]: partitions [p0, p0+np) of a
    group tile, source columns [r0, r0+L), packed dest column col0.
    coarse=True loads group unions (fills every packed slot — used for the
    first pool rotation so no uninitialized SBUF is ever read)."""
    out = []
    for g in range(NG):
        if coarse:
            halves = [(0, 128, GRP_P[g], GRP_SS[g])]
        else:
            halves = [
                (h * 64, 64, SUB_P[2 * g + h], SUB_SS[2 * g + h]) for h in (0, 1)
            ]
        for p0, npart, plen, ss in halves:
            if ss <= plen + 1:
                out.append((g, p0, npart, 0, R, _col_of(g, 0)))
            else:
                out.append((g, p0, npart, 0, plen, _col_of(g, 0)))
                out.append((g, p0, npart, ss, R - ss, _col_of(g, ss)))
    return out


PIECES_FINE = _pieces(False)
PIECES_COARSE = _pieces(True)


def _mask_qr() -> np.ndarray:
    """[512, 513] f64: wanted(q, r) / (C * diag_len)."""
    q = np.arange(H, dtype=np.int64)[:, None]
    r = np.arange(R, dtype=np.int64)[None, :]
    prefix = (r <= H // 2) & (q + r <= H - 1)            # diagonal o = r
    suffix = (r > H // 2) & (q + r >= H) & (q <= H - 2)  # o = r - 513
    mask = prefix | suffix
    o = np.where(r <= H // 2, r, r - R)
    denom = float(C) * (H - np.abs(o)).astype(np.float64)
    return mask.astype(np.float64) / denom


def _build_weights() -> np.ndarray:
    """[128, FP] f32 in the packed SBUF layout."""
    wqr = _mask_qr()
    wp = np.zeros((128, FP), dtype=np.float64)
    for g in range(NG):
        off = GRP_BASE[g]
        for r0, L in GSEGS[g]:
            wp[:, off : off + L] = wqr[g * 128 : (g + 1) * 128, r0 : r0 + L]
            off += L
    return wp.astype(np.float32)


def _build_program():
    nc = bacc.Bacc("TRN2", target_bir_lowering=False, debug=False, num_devices=B)
    xp = nc.dram_tensor("x", [N_IN], F32, kind="ExternalInput")
    wt = nc.dram_tensor("w", [128, FP], F32, kind="ExternalInput")
    out_t = nc.dram_tensor("out", [1, R], F32, kind="ExternalOutput")

    NBUFS = 10

    with tile.TileContext(nc) as tc:
        with (
            tc.tile_pool(name="consts", bufs=1) as consts,
            tc.tile_pool(name="accp", bufs=1) as accp,
            tc.tile_pool(name="loadp", bufs=NBUFS) as loadp,
            tc.tile_pool(name="outp", bufs=1) as outp,
            tc.tile_pool(name="psum", bufs=2, space=bass.MemorySpace.PSUM) as psump,
        ):
            w_tile = consts.tile([128, FP], F32)
            nc.sync.dma_start(out=w_tile[:], in_=wt.ap())
            ones = consts.tile([128, 1], F32)
            nc.vector.memset(ones[:], 1.0)

            acc = accp.tile([128, FP], F32)
            ndma = 0
            for c in range(C):
                t = loadp.tile([128, FP], F32)
                # first pool rotation: fill every packed slot (no uninit reads)
                pieces = PIECES_COARSE if c < NBUFS else PIECES_FINE
                for g, p0, npart, r0, L, col0 in pieces:
                    src = bass.AP(
                        xp,
                        c * CH_ELEMS + R * (g * 128 + p0) + r0,
                        [[R, npart], [1, L]],
                    )
                    eng = nc.sync if ndma % 2 == 0 else nc.scalar
                    eng.dma_start(out=t[p0 : p0 + npart, col0 : col0 + L], in_=src)
                    ndma += 1
                if c == 0:
                    nc.vector.tensor_copy(out=acc[:], in_=t[:])
                else:
                    nc.vector.tensor_add(out=acc[:], in0=acc[:], in1=t[:])

            # mask + 1/denominator, then fold the 4 groups into r-order
            nc.vector.tensor_mul(out=acc[:], in0=acc[:], in1=w_tile[:])
            u = outp.tile([128, R], F32)
            first = True
            for g in range(NG):
                off = GRP_BASE[g]
                for r0, L in GSEGS[g]:
                    if first:
                        # g0 seg A is (0, 257): start with a full-width memset
                        nc.vector.memset(u[:], 0.0)
                        nc.vector.tensor_add(
                            out=u[:, r0 : r0 + L],
                            in0=u[:, r0 : r0 + L],
                            in1=acc[:, off : off + L],
                        )
                        first = False
                    else:
                        nc.vector.tensor_add(
                            out=u[:, r0 : r0 + L],
                            in0=u[:, r0 : r0 + L],
                            in1=acc[:, off : off + L],
                        )
                    off += L

            # partition reduction: ones[128,1]^T @ u[128, N] -> [1, N]
            ps_a = psump.tile([1, 512], F32)
            ps_b = psump.tile([1, 1], F32)
            nc.tensor.matmul(ps_a[:], ones[:], u[:, 0:512], start=True, stop=True)
            nc.tensor.matmul(ps_b[:], ones[:], u[:, 512:513], start=True, stop=True)
            res = outp.tile([1, R], F32)
            nc.vector.tensor_copy(out=res[:, 0:512], in_=ps_a[:])
            nc.vector.tensor_copy(out=res[:, 512:513], in_=ps_b[:])
            nc.sync.dma_start(out=out_t.ap(), in_=res[:])

    nc.compile()
    return nc


_CACHE = {}


def kernel(x, _trace=False, _trace_cores=None) -> np.ndarray:
    x = np.asarray(x, dtype=np.float32)
    assert x.shape == (B, C, H, H), x.shape

    if "nc" not in _CACHE:
        _CACHE["nc"] = _build_program()
        _CACHE["w"] = _build_weights()
    nc = _CACHE["nc"]
    w = _CACHE["w"]

    pad = np.zeros(PAD, dtype=np.float32)
    in_maps = [
        {"x": np.concatenate([np.ascontiguousarray(x[b]).reshape(-1), pad]), "w": w}
        for b in range(B)
    ]
    result = run_bass_kernel_spmd(
        nc,
        in_maps,
        core_ids=list(range(B)),
        trace=_trace,
        trace_cores=_trace_cores,
    )
    _CACHE["last_result"] = result

    out = np.empty((B, 1, R), dtype=np.float32)
    for b in range(B):
        r = result.results[b]["out"].reshape(R)
        # column r -> offset o = r (r <= 256) / r - 513 (r >= 257);
        # output index n = o + 256
        out[b, 0, :] = np.concatenate([r[R - 256 :], r[: R - 256]])
    return out


# revision 5
# speedup vs baseline: 1.3383x; 1.3383x over previous
"""DiagPooling (segment-reduce over square-image diagonals) on 8 NeuronCores.

Input  x: [8, 128, 512, 512] f32. Output: [8, 1, 513] f32 — per batch, the
mean over (channels, diagonal) of each diagonal offset in [-256, 256].

Sharding: batch b -> core b (data parallel, no communication).

Per-core algorithm (stride-513 trick): for a flat 512x512 image, element
(i, j) sits at a = 512*i + j = 513*i + (j - i). Viewing the flat image as
rows of 513 (an overlapping strided view), every diagonal becomes a COLUMN:
P[q, r] = flat[513*q + r] holds diagonal o = r (when q + r <= 511) or
o = r - 513 (when q + r >= 512). The wanted diagonals o in [-256, 256] are
exactly {r <= 256, q <= 511 - r} and {r >= 257, q >= 512 - r}; a fixed mask
folded with 1/(C*diag_len) turns the segment reduce into masked column sums.
Channels are summed first (the mask is channel-invariant), so the mask is
applied once to ~1 MiB instead of 128 MiB.

Banding: each 64-row band of P only needs a prefix [0, Plen) and a suffix
[SS, 513) of columns (the rest of the row lies on diagonals outside
[-256, 256]), so the DMA skips ~19% of HBM traffic. Unwanted elements that
do get loaded (band unions, merged holes) are zeroed by the mask.

SBUF layout: P row q = g*128 + p -> partition p, group g. Each group's live
columns are packed side by side (width 385/513/513/384, total 1795), which
also shortens the per-channel DVE accumulate.
"""

import numpy as np

import concourse.bass as bass
import concourse.bacc as bacc
import concourse.mybir as mybir
from concourse import tile
from concourse.bass_utils import run_bass_kernel_spmd

B, C, H = 8, 128, 512
R = H + 1               # 513: columns of the strided view
NG = 4                  # 512 q-rows -> 4 groups of 128 partitions
CH_ELEMS = H * H        # elements per (b, c) image
PAD = H                 # tail pad so the last P-row read stays in bounds
N_IN = C * CH_ELEMS + PAD
F32 = mybir.dt.float32

# Per 64-row subgroup s (q in [64s, 64s+64)): wanted columns are
# [0, SUB_P[s]) and [SUB_SS[s], 513).
SUB_P = [257, 257, 257, 257, 256, 192, 128, 64]
SUB_SS = [449, 385, 321, 257, 257, 257, 257, 257]
# Group unions (128 rows): prefix len / suffix start.
GRP_P = [max(SUB_P[2 * g], SUB_P[2 * g + 1]) for g in range(NG)]
GRP_SS = [min(SUB_SS[2 * g], SUB_SS[2 * g + 1]) for g in range(NG)]


def _group_segs():
    """Per group: list of (r0, length) packed consecutively. A group whose
    prefix and suffix (almost) touch is merged into one full segment."""
    segs = []
    for g in range(NG):
        if GRP_SS[g] <= GRP_P[g] + 1:
            segs.append([(0, R)])
        else:
            segs.append([(0, GRP_P[g]), (GRP_SS[g], R - GRP_SS[g])])
    return segs


GSEGS = _group_segs()
GRP_W = [sum(L for _, L in s) for s in GSEGS]       # [385, 513, 513, 384]
GRP_BASE = [sum(GRP_W[:g]) for g in range(NG)]
FP = sum(GRP_W)                                     # 1795 packed columns


def _col_of(g, r):
    """Packed column of (group g, column r). r must lie in a group segment."""
    off = GRP_BASE[g]
    for r0, L in GSEGS[g]:
        if r0 <= r < r0 + L:
            return off + (r - r0)
        off += L
    raise ValueError((g, r))


def _pieces(coarse: bool):
    """DMA piece list [(p0, np, r0, L, col0)]: partitions [p0, p0+np) of a
    group tile, source columns [r0, r0+L), packed dest column col0.
    coarse=True loads group unions (fills every packed slot — used for the
    first pool rotation so no uninitialized SBUF is ever read)."""
    out = []
    for g in range(NG):
        if coarse:
            halves = [(0, 128, GRP_P[g], GRP_SS[g])]
        else:
            halves = [
                (h * 64, 64, SUB_P[2 * g + h], SUB_SS[2 * g + h]) for h in (0, 1)
            ]
        for p0, npart, plen, ss in halves:
            if ss <= plen + 1:
                out.append((g, p0, npart, 0, R, _col_of(g, 0)))
            else:
                out.append((g, p0, npart, 0, plen, _col_of(g, 0)))
                out.append((g, p0, npart, ss, R - ss, _col_of(g, ss)))
    return out


PIECES_FINE = _pieces(False)
PIECES_COARSE = _pieces(True)


def _mask_qr() -> np.ndarray:
    """[512, 513] f64: wanted(q, r) / (C * diag_len)."""
    q = np.arange(H, dtype=np.int64)[:, None]
    r = np.arange(R, dtype=np.int64)[None, :]
    prefix = (r <= H // 2) & (q + r <= H - 1)            # diagonal o = r
    suffix = (r > H // 2) & (q + r >= H) & (q <= H - 2)  # o = r - 513
    mask = prefix | suffix
    o = np.where(r <= H // 2, r, r - R)
    denom = float(C) * (H - np.abs(o)).astype(np.float64)
    return mask.astype(np.float64) / denom


def _build_weights() -> np.ndarray:
    """[128, FP] f32 in the packed SBUF layout."""
    wqr = _mask_qr()
    wp = np.zeros((128, FP), dtype=np.float64)
    for g in range(NG):
        off = GRP_BASE[g]
        for r0, L in GSEGS[g]:
            wp[:, off : off + L] = wqr[g * 128 : (g + 1) * 128, r0 : r0 + L]
            off += L
    return wp.astype(np.float32)


def _build_program():
    nc = bacc.Bacc("TRN2", target_bir_lowering=False, debug=False, num_devices=B)
    xp = nc.dram_tensor("x", [N_IN], F32, kind="ExternalInput")
    wt = nc.dram_tensor("w", [128, FP], F32, kind="ExternalInput")
    out_t = nc.dram_tensor("out", [1, R], F32, kind="ExternalOutput")

    NBUFS = 2
    K = 8  # channels per super-tile: batches piece DMAs 8x (3rd AP dim)
    assert C % K == 0

    with tile.TileContext(nc) as tc:
        with (
            tc.tile_pool(name="consts", bufs=1) as consts,
            tc.tile_pool(name="accp", bufs=1) as accp,
            tc.tile_pool(name="loadp", bufs=NBUFS) as loadp,
            tc.tile_pool(name="outp", bufs=1) as outp,
            tc.tile_pool(name="psum", bufs=2, space=bass.MemorySpace.PSUM) as psump,
        ):
            w_tile = consts.tile([128, FP], F32)
            nc.sync.dma_start(out=w_tile[:], in_=wt.ap())
            ones = consts.tile([128, 1], F32)
            nc.vector.memset(ones[:], 1.0)

            acc = accp.tile([128, FP], F32)
            ndma = 0
            for c0 in range(0, C, K):
                t = loadp.tile([128, K * FP], F32)
                t3 = t[:].rearrange("p (k f) -> p k f", k=K)
                # first pool rotation: fill every packed slot (no uninit reads)
                pieces = PIECES_COARSE if c0 < NBUFS * K else PIECES_FINE
                for g, p0, npart, r0, L, col0 in pieces:
                    src = bass.AP(
                        xp,
                        c0 * CH_ELEMS + R * (g * 128 + p0) + r0,
                        [[R, npart], [CH_ELEMS, K], [1, L]],
                    )
                    eng = nc.sync if ndma % 2 == 0 else nc.scalar
                    eng.dma_start(
                        out=t3[p0 : p0 + npart, :, col0 : col0 + L], in_=src
                    )
                    ndma += 1
                for k in range(K):
                    sl = t[:, k * FP : (k + 1) * FP]
                    if c0 + k == 0:
                        nc.vector.tensor_copy(out=acc[:], in_=sl)
                    else:
                        nc.vector.tensor_add(out=acc[:], in0=acc[:], in1=sl)

            # mask + 1/denominator, then fold the 4 groups into r-order
            nc.vector.tensor_mul(out=acc[:], in0=acc[:], in1=w_tile[:])
            u = outp.tile([128, R], F32)
            first = True
            for g in range(NG):
                off = GRP_BASE[g]
                for r0, L in GSEGS[g]:
                    if first:
                        # g0 seg A is (0, 257): start with a full-width memset
                        nc.vector.memset(u[:], 0.0)
                        nc.vector.tensor_add(
                            out=u[:, r0 : r0 + L],
                            in0=u[:, r0 : r0 + L],
                            in1=acc[:, off : off + L],
                        )
                        first = False
                    else:
                        nc.vector.tensor_add(
                            out=u[:, r0 : r0 + L],
                            in0=u[:, r0 : r0 + L],
                            in1=acc[:, off : off + L],
                        )
                    off += L

            # partition reduction: ones[128,1]^T @ u[128, N] -> [1, N]
            ps_a = psump.tile([1, 512], F32)
            ps_b = psump.tile([1, 1], F32)
            nc.tensor.matmul(ps_a[:], ones[:], u[:, 0:512], start=True, stop=True)
            nc.tensor.matmul(ps_b[:], ones[:], u[:, 512:513], start=True, stop=True)
            res = outp.tile([1, R], F32)
            nc.vector.tensor_copy(out=res[:, 0:512], in_=ps_a[:])
            nc.vector.tensor_copy(out=res[:, 512:513], in_=ps_b[:])
            nc.sync.dma_start(out=out_t.ap(), in_=res[:])

    nc.compile()
    return nc


_CACHE = {}


def kernel(x, _trace=False, _trace_cores=None) -> np.ndarray:
    x = np.asarray(x, dtype=np.float32)
    assert x.shape == (B, C, H, H), x.shape

    if "nc" not in _CACHE:
        _CACHE["nc"] = _build_program()
        _CACHE["w"] = _build_weights()
    nc = _CACHE["nc"]
    w = _CACHE["w"]

    pad = np.zeros(PAD, dtype=np.float32)
    in_maps = [
        {"x": np.concatenate([np.ascontiguousarray(x[b]).reshape(-1), pad]), "w": w}
        for b in range(B)
    ]
    result = run_bass_kernel_spmd(
        nc,
        in_maps,
        core_ids=list(range(B)),
        trace=_trace,
        trace_cores=_trace_cores,
    )
    _CACHE["last_result"] = result

    out = np.empty((B, 1, R), dtype=np.float32)
    for b in range(B):
        r = result.results[b]["out"].reshape(R)
        # column r -> offset o = r (r <= 256) / r - 513 (r >= 257);
        # output index n = o + 256
        out[b, 0, :] = np.concatenate([r[R - 256 :], r[: R - 256]])
    return out


# revision 6
# speedup vs baseline: 1.8152x; 1.3563x over previous
"""DiagPooling (segment-reduce over square-image diagonals) on 8 NeuronCores.

Input  x: [8, 128, 512, 512] f32. Output: [8, 1, 513] f32 — per batch, the
mean over (channels, diagonal) of each diagonal offset in [-256, 256].

Sharding: batch b -> core b (data parallel, no communication).

Per-core algorithm (stride-513 trick): for a flat 512x512 image, element
(i, j) sits at a = 512*i + j = 513*i + (j - i). Viewing the flat image as
rows of 513 (an overlapping strided view), every diagonal becomes a COLUMN:
P[q, r] = flat[513*q + r] holds diagonal o = r (when q + r <= 511) or
o = r - 513 (when q + r >= 512). The wanted diagonals o in [-256, 256] are
exactly {r <= 256, q <= 511 - r} and {r >= 257, q >= 512 - r}; a fixed mask
folded with 1/(C*diag_len) turns the segment reduce into masked column sums.
Channels are summed first (the mask is channel-invariant), so the mask is
applied once to ~1 MiB instead of 128 MiB.

Banding: each 64-row band of P only needs a column prefix [0, P_s) and
suffix [SS_s, 513) (the rest lies on diagonals outside [-256, 256]), which
skips ~19% of HBM traffic. To keep DMA descriptors large, each loaded chunk
WRAPS a row boundary: chunk(q) = flat[513q - (513-SS), 513q + P) — row q-1's
suffix merged with row q's prefix, one contiguous 1.3-2 KB run per row.
A slot's column still maps to a fixed diagonal r (only the contributing row
shifts by one for suffix columns), so the precomputed mask handles holes,
band over-approximation, and padding garbage uniformly.

SBUF layout: P row q = g*128 + p -> partition p, group g; group layouts are
packed side by side (widths 385/513/513/384, total 1795), which also
shortens the per-channel DVE accumulate. Channels are loaded 8 at a time so
each of the 8 wrapped band pieces is a single ~1 MB DMA.
"""

import numpy as np

import concourse.bass as bass
import concourse.bacc as bacc
import concourse.mybir as mybir
from concourse import tile
from concourse.bass_utils import run_bass_kernel_spmd

B, C, H = 8, 128, 512
R = H + 1               # 513: columns of the strided view
NG = 4                  # 512 q-rows -> 4 groups of 128 partitions
CH_ELEMS = H * H        # elements per (b, c) image
FRONT_PAD = H           # so wrapped chunks of row 0 stay in bounds
TAIL_PAD = H            # so the last P-row read stays in bounds
N_IN = FRONT_PAD + C * CH_ELEMS + TAIL_PAD
F32 = mybir.dt.float32

# Per 64-row subgroup s (q in [64s, 64s+64)): wanted columns are
# [0, SUB_P[s]) and [SUB_SS[s], 513).
SUB_P = [257, 257, 257, 257, 256, 192, 128, 64]
SUB_SS = [449, 385, 321, 257, 257, 257, 257, 257]
# Group unions (128 rows): prefix len / suffix start / wrapped suffix width.
GRP_P = [max(SUB_P[2 * g], SUB_P[2 * g + 1]) for g in range(NG)]
GRP_SS = [min(SUB_SS[2 * g], SUB_SS[2 * g + 1]) for g in range(NG)]
GRP_WS = [R - ss for ss in GRP_SS]
GRP_W = [GRP_WS[g] + GRP_P[g] for g in range(NG)]   # [385, 513, 513, 384]
GRP_BASE = [sum(GRP_W[:g]) for g in range(NG)]
FP = sum(GRP_W)                                     # 1795 packed columns


def _pieces(coarse: bool):
    """Wrapped DMA pieces [(p0, npart, flat_off, CL, col0)] per channel:
    partitions [p0, p0+npart), chunk flat[513*q - (513-SS), 513*q + P) for
    each row q, length CL, packed dest column col0. coarse=True uses group
    unions on all 128 partitions (fills every packed slot — used for the
    first pool rotation so no uninitialized SBUF is ever read)."""
    out = []
    for g in range(NG):
        if coarse:
            halves = [(0, 128, GRP_P[g], GRP_SS[g])]
        else:
            halves = [
                (h * 64, 64, SUB_P[2 * g + h], SUB_SS[2 * g + h]) for h in (0, 1)
            ]
        for p0, npart, plen, ss in halves:
            cl = (R - ss) + plen
            q0 = g * 128 + p0
            flat_off = R * q0 - (R - ss)
            col0 = GRP_BASE[g] + (ss - GRP_SS[g])
            out.append((p0, npart, flat_off, cl, col0))
    return out


PIECES_FINE = _pieces(False)
PIECES_COARSE = _pieces(True)


def _mask_qr() -> np.ndarray:
    """[512, 513] f64: wanted(q, r) / (C * diag_len); row -1 handled by pad."""
    q = np.arange(H, dtype=np.int64)[:, None]
    r = np.arange(R, dtype=np.int64)[None, :]
    prefix = (r <= H // 2) & (q + r <= H - 1)            # diagonal o = r
    suffix = (r > H // 2) & (q + r >= H) & (q <= H - 2)  # o = r - 513
    mask = prefix | suffix
    o = np.where(r <= H // 2, r, r - R)
    denom = float(C) * (H - np.abs(o)).astype(np.float64)
    return mask.astype(np.float64) / denom


def _build_weights() -> np.ndarray:
    """[128, FP] f32 in the packed wrapped layout: suffix columns hold row
    q-1, prefix columns hold row q."""
    wqr = _mask_qr()
    wp = np.zeros((128, FP), dtype=np.float64)
    for g in range(NG):
        b0, ws, gp, gss = GRP_BASE[g], GRP_WS[g], GRP_P[g], GRP_SS[g]
        for p in range(128):
            q = g * 128 + p
            if q >= 1:
                wp[p, b0 : b0 + ws] = wqr[q - 1, gss:R]
            wp[p, b0 + ws : b0 + ws + gp] = wqr[q, 0:gp]
    return wp.astype(np.float32)


def _build_program():
    nc = bacc.Bacc("TRN2", target_bir_lowering=False, debug=False, num_devices=B)
    xp = nc.dram_tensor("x", [N_IN], F32, kind="ExternalInput")
    wt = nc.dram_tensor("w", [128, FP], F32, kind="ExternalInput")
    out_t = nc.dram_tensor("out", [1, R], F32, kind="ExternalOutput")

    NBUFS = 2
    K = 8  # channels per super-tile: batches piece DMAs 8x (3rd AP dim)
    assert C % K == 0

    with tile.TileContext(nc) as tc:
        with (
            tc.tile_pool(name="consts", bufs=1) as consts,
            tc.tile_pool(name="accp", bufs=1) as accp,
            tc.tile_pool(name="loadp", bufs=NBUFS) as loadp,
            tc.tile_pool(name="outp", bufs=1) as outp,
            tc.tile_pool(name="psum", bufs=2, space=bass.MemorySpace.PSUM) as psump,
        ):
            w_tile = consts.tile([128, FP], F32)
            nc.sync.dma_start(out=w_tile[:], in_=wt.ap())
            ones = consts.tile([128, 1], F32)
            nc.vector.memset(ones[:], 1.0)

            acc = accp.tile([128, FP], F32)
            ndma = 0
            for c0 in range(0, C, K):
                t = loadp.tile([128, K * FP], F32)
                t3 = t[:].rearrange("p (k f) -> p k f", k=K)
                # first pool rotation: fill every packed slot (no uninit reads)
                pieces = PIECES_COARSE if c0 < NBUFS * K else PIECES_FINE
                for p0, npart, flat_off, cl, col0 in pieces:
                    src = bass.AP(
                        xp,
                        FRONT_PAD + c0 * CH_ELEMS + flat_off,
                        [[R, npart], [CH_ELEMS, K], [1, cl]],
                    )
                    eng = nc.sync if ndma % 2 == 0 else nc.scalar
                    eng.dma_start(
                        out=t3[p0 : p0 + npart, :, col0 : col0 + cl], in_=src
                    )
                    ndma += 1
                for k in range(K):
                    sl = t[:, k * FP : (k + 1) * FP]
                    if c0 + k == 0:
                        nc.vector.tensor_copy(out=acc[:], in_=sl)
                    else:
                        nc.vector.tensor_add(out=acc[:], in0=acc[:], in1=sl)

            # mask + 1/denominator, then fold the 4 groups into r-order
            nc.vector.tensor_mul(out=acc[:], in0=acc[:], in1=w_tile[:])
            u = outp.tile([128, R], F32)
            nc.vector.memset(u[:], 0.0)
            for g in range(NG):
                b0, ws, gp, gss = GRP_BASE[g], GRP_WS[g], GRP_P[g], GRP_SS[g]
                nc.vector.tensor_add(
                    out=u[:, gss:R], in0=u[:, gss:R], in1=acc[:, b0 : b0 + ws]
                )
                nc.vector.tensor_add(
                    out=u[:, 0:gp],
                    in0=u[:, 0:gp],
                    in1=acc[:, b0 + ws : b0 + ws + gp],
                )

            # partition reduction: ones[128,1]^T @ u[128, N] -> [1, N]
            ps_a = psump.tile([1, 512], F32)
            ps_b = psump.tile([1, 1], F32)
            nc.tensor.matmul(ps_a[:], ones[:], u[:, 0:512], start=True, stop=True)
            nc.tensor.matmul(ps_b[:], ones[:], u[:, 512:513], start=True, stop=True)
            res = outp.tile([1, R], F32)
            nc.vector.tensor_copy(out=res[:, 0:512], in_=ps_a[:])
            nc.vector.tensor_copy(out=res[:, 512:513], in_=ps_b[:])
            nc.sync.dma_start(out=out_t.ap(), in_=res[:])

    nc.compile()
    return nc


_CACHE = {}


def kernel(x, _trace=False, _trace_cores=None) -> np.ndarray:
    x = np.asarray(x, dtype=np.float32)
    assert x.shape == (B, C, H, H), x.shape

    if "nc" not in _CACHE:
        _CACHE["nc"] = _build_program()
        _CACHE["w"] = _build_weights()
    nc = _CACHE["nc"]
    w = _CACHE["w"]

    fpad = np.zeros(FRONT_PAD, dtype=np.float32)
    tpad = np.zeros(TAIL_PAD, dtype=np.float32)
    in_maps = [
        {
            "x": np.concatenate(
                [fpad, np.ascontiguousarray(x[b]).reshape(-1), tpad]
            ),
            "w": w,
        }
        for b in range(B)
    ]
    result = run_bass_kernel_spmd(
        nc,
        in_maps,
        core_ids=list(range(B)),
        trace=_trace,
        trace_cores=_trace_cores,
    )
    _CACHE["last_result"] = result

    out = np.empty((B, 1, R), dtype=np.float32)
    for b in range(B):
        r = result.results[b]["out"].reshape(R)
        # column r -> offset o = r (r <= 256) / r - 513 (r >= 257);
        # output index n = o + 256
        out[b, 0, :] = np.concatenate([r[R - 256 :], r[: R - 256]])
    return out


# revision 7
# speedup vs baseline: 1.9858x; 1.0940x over previous
"""DiagPooling (segment-reduce over square-image diagonals) on 8 NeuronCores.

Input  x: [8, 128, 512, 512] f32. Output: [8, 1, 513] f32 — per batch, the
mean over (channels, diagonal) of each diagonal offset in [-256, 256].

Sharding: batch b -> core b (data parallel, no communication).

Per-core pipeline:
1. Stream the 128 channels in their natural contiguous layout ([128, 2048]
   tiles, 8 KB per partition per DMA descriptor — the fastest possible HBM
   pattern) and accumulate on VectorE: y = sum_c x[b, c]. The channel sum is
   layout-agnostic, so the expensive 128 MiB stream needs no rearrangement.
2. Re-lay the reduced y (1 MiB) via a DRAM round-trip into the stride-513
   view: P[q, r] = y_flat[513*q + r]. Since flat (i, j) = 513*i + (j - i),
   every diagonal of y is a COLUMN of P: column r holds diagonal o = r
   (rows q <= 511 - r) and o = r - 513 (rows q >= 512 - r).
3. The wanted diagonals o in [-256, 256] are selected by a precomputed mask
   folded with 1/(C*diag_len); masked column sums (one elementwise multiply,
   a 4-group fold, and a ones-vector matmul over partitions) give the means.
"""

import numpy as np

import concourse.bass as bass
import concourse.bacc as bacc
import concourse.mybir as mybir
from concourse import tile
from concourse.bass_utils import run_bass_kernel_spmd

B, C, H = 8, 128, 512
R = H + 1               # 513: columns of the strided view
NG = 4                  # 512 q-rows -> 4 groups of 128 partitions
F = NG * R              # 2052: SBUF free width of the strided view
CH_ELEMS = H * H        # elements per (b, c) image
FW = CH_ELEMS // 128    # 2048: flat free width per partition
N_IN = C * CH_ELEMS
Y_PAD = CH_ELEMS + H    # 262656 = 512*513: padded scratch for the P view
F32 = mybir.dt.float32


def _mask_qr() -> np.ndarray:
    """[512, 513] f64: wanted(q, r) / (C * diag_len)."""
    q = np.arange(H, dtype=np.int64)[:, None]
    r = np.arange(R, dtype=np.int64)[None, :]
    prefix = (r <= H // 2) & (q + r <= H - 1)            # diagonal o = r
    suffix = (r > H // 2) & (q + r >= H) & (q <= H - 2)  # o = r - 513
    mask = prefix | suffix
    o = np.where(r <= H // 2, r, r - R)
    denom = float(C) * (H - np.abs(o)).astype(np.float64)
    return mask.astype(np.float64) / denom


def _build_weights() -> np.ndarray:
    """[128, F] f32: the mask in the SBUF strided-view layout
    (row q = g*128 + p -> partition p, free column g*513 + r)."""
    wqr = _mask_qr()
    return (
        wqr.reshape(NG, 128, R).transpose(1, 0, 2).reshape(128, F).astype(np.float32)
    )


def _build_program():
    nc = bacc.Bacc("TRN2", target_bir_lowering=False, debug=False, num_devices=B)
    xp = nc.dram_tensor("x", [N_IN], F32, kind="ExternalInput")
    wt = nc.dram_tensor("w", [128, F], F32, kind="ExternalInput")
    out_t = nc.dram_tensor("out", [1, R], F32, kind="ExternalOutput")
    y_dram = nc.dram_tensor("y_scratch", [Y_PAD], F32)

    NBUFS = 12

    with tile.TileContext(nc) as tc:
        with (
            tc.tile_pool(name="consts", bufs=1) as consts,
            tc.tile_pool(name="accp", bufs=1) as accp,
            tc.tile_pool(name="loadp", bufs=NBUFS) as loadp,
            tc.tile_pool(name="outp", bufs=1) as outp,
            tc.tile_pool(name="psum", bufs=2, space=bass.MemorySpace.PSUM) as psump,
        ):
            w_tile = consts.tile([128, F], F32)
            nc.sync.dma_start(out=w_tile[:], in_=wt.ap())
            ones = consts.tile([128, 1], F32)
            nc.vector.memset(ones[:], 1.0)
            # zero the scratch tail so the strided re-read never sees junk
            zpad = consts.tile([1, H], F32)
            nc.vector.memset(zpad[:], 0.0)
            nc.sync.dma_start(
                out=bass.AP(y_dram, CH_ELEMS, [[1, H]]), in_=zpad[:]
            )

            # 1. contiguous channel stream, flat accumulate
            acc = accp.tile([128, FW], F32)
            for c in range(C):
                t = loadp.tile([128, FW], F32)
                nc.sync.dma_start(
                    out=t[:], in_=bass.AP(xp, c * CH_ELEMS, [[FW, 128], [1, FW]])
                )
                if c == 0:
                    nc.vector.tensor_copy(out=acc[:], in_=t[:])
                else:
                    nc.vector.tensor_add(out=acc[:], in0=acc[:], in1=t[:])

            # 2. re-layout y through DRAM into the stride-513 view
            nc.sync.dma_start(
                out=bass.AP(y_dram, 0, [[FW, 128], [1, FW]]), in_=acc[:]
            )
            p_tile = outp.tile([128, F], F32)
            nc.scalar.dma_start(
                out=p_tile[:].rearrange("p (g r) -> p g r", g=NG),
                in_=bass.AP(y_dram, 0, [[R, 128], [R * 128, NG], [1, R]]),
            )

            # 3. masked column sums
            nc.vector.tensor_mul(out=p_tile[:], in0=p_tile[:], in1=w_tile[:])
            u = outp.tile([128, R], F32)
            nc.vector.tensor_add(
                out=u[:], in0=p_tile[:, 0:R], in1=p_tile[:, R : 2 * R]
            )
            nc.vector.tensor_add(out=u[:], in0=u[:], in1=p_tile[:, 2 * R : 3 * R])
            nc.vector.tensor_add(out=u[:], in0=u[:], in1=p_tile[:, 3 * R : 4 * R])

            # partition reduction: ones[128,1]^T @ u[128, N] -> [1, N]
            ps_a = psump.tile([1, 512], F32)
            ps_b = psump.tile([1, 1], F32)
            nc.tensor.matmul(ps_a[:], ones[:], u[:, 0:512], start=True, stop=True)
            nc.tensor.matmul(ps_b[:], ones[:], u[:, 512:513], start=True, stop=True)
            res = outp.tile([1, R], F32)
            nc.vector.tensor_copy(out=res[:, 0:512], in_=ps_a[:])
            nc.vector.tensor_copy(out=res[:, 512:513], in_=ps_b[:])
            nc.sync.dma_start(out=out_t.ap(), in_=res[:])

    nc.compile()
    return nc


_CACHE = {}


def kernel(x, _trace=False, _trace_cores=None) -> np.ndarray:
    x = np.asarray(x, dtype=np.float32)
    assert x.shape == (B, C, H, H), x.shape

    if "nc" not in _CACHE:
        _CACHE["nc"] = _build_program()
        _CACHE["w"] = _build_weights()
    nc = _CACHE["nc"]
    w = _CACHE["w"]

    in_maps = [
        {"x": np.ascontiguousarray(x[b]).reshape(-1), "w": w} for b in range(B)
    ]
    result = run_bass_kernel_spmd(
        nc,
        in_maps,
        core_ids=list(range(B)),
        trace=_trace,
        trace_cores=_trace_cores,
    )
    _CACHE["last_result"] = result

    out = np.empty((B, 1, R), dtype=np.float32)
    for b in range(B):
        r = result.results[b]["out"].reshape(R)
        # column r -> offset o = r (r <= 256) / r - 513 (r >= 257);
        # output index n = o + 256
        out[b, 0, :] = np.concatenate([r[R - 256 :], r[: R - 256]])
    return out
